# revision 30
# baseline (speedup 1.0000x reference)
"""YOLO-style loss (nn_Loss_90142773608781) on 8 Trainium2 NeuronCores.

Strategy (data-parallel, host-side sharding + gather):
- Cells sharded by batch range: core c owns cells [c*100352, (c+1)*100352).
  Targets follow their cell's core (batch_id // 2048).
- The host gathers each target's 30-float grid row (pure data movement)
  and builds one dense per-core bf16 tile in a dim-major SoA layout
  ([x0,x1,y0,y1], [w0,w1,h0,h1], ...) so every DVE op is unit-stride;
  one big load (4.2KB per-partition lines) instead of many small ones.
  Target-side fields (signed sqrts, box edges, areas/4) are precomputed
  on host and duplicated per box lane to keep packed bf16 DVE modes.
- On device each core runs ONE full-width pass over its 9216 slots
  (72 per partition): IoU cross-multiply box select, then *masked
  accumulation* - every per-target term is computed for BOTH boxes and
  summed with the 0/1 responsibility mask, so there is no box-gather.
  The iou guard drops out: ain>0 implies atot >= area_t >> 1e-6, so only
  u=max(atot,eps) is needed; the whole area algebra runs at 1/4 scale
  (host tab/4, wih relu fused with *0.5) which the cross-multiply
  comparison is invariant to.
- Padding slots are all-zero; their only residue is the obj term's
  0.5*(0-2)^2 = 2 per pad, corrected on host.
- The obj term rides the coord/size Square-accumulate: masked diffs and
  the masked (c-2)/sqrt(10) live in one [P,W,10] layout reduced by
  activation(Square, scale=sqrt(5), accum_out), split in two halves so
  the Scalar engine can start while the DVE finishes the second half.
- abs/sign for the signed sqrt are bf16 bit ops on the DVE (and 0x7fff /
  and 0x8000 + or), keeping ACT to Sqrt/Square/Copy (one table set).
- conf and cls-grid squares ride fp8 (e4m3): random rounding cancels
  across 1.6M/1.3M terms; the ~0.1% systematic square bias is far below
  the 2e-2 gate.
- Host reduces the [P,7] partials; constants: obj identity -NTGT and
  cls_r identity +NTGT cancel; pad obj residue -2*8192 remains.
"""

import sys

if "/opt/trn_rl_repo" not in sys.path:
    sys.path.append("/opt/trn_rl_repo")

import numpy as np
import ml_dtypes

P = 128
W = 72                    # slots per partition
NS = P * W                # 9216 slots per core
GRID = 7
BATCH = 16384
NTGT = 65536
CELLS = BATCH * GRID * GRID
CELLS_CORE = CELLS // 8   # 100352
CONF_W = CELLS_CORE * 2 // P   # 1568
PAD_TOT = 8 * NS - NTGT   # 8192

RT5 = 2.2360679774997896   # sqrt(5)
RT2I = 0.7071067811865476  # sqrt(0.5)
RT10I = 0.31622776601683794  # 1/sqrt(10)

# grp1 blocks (units of W columns):
# [gwh 0:4][gxy 4:8][lt 8:12][rb 12:16][txy 16:20][tssq 20:24]
# [tab4 24:26][clsr 26:27][gc 27:29]
G1W = 29 * W

_cache = {}


def _build():
    import concourse.bacc as bacc
    import concourse.tile as tile
    import concourse.mybir as mybir
    from concourse import hw_specs

    # The act-table-load pass picks, per activation, the FIRST set in
    # act_info.json containing its function; square/copy then resolve to
    # set 0 while sqrt needs set 3 -> two ~1.3us table loads. Blanking
    # every set except sqrt_and_others (indices preserved) makes all our
    # functions (sqrt/square/copy live there too) resolve to one set.
    orig_tables = hw_specs.get_activation_tables

    def _one_set(arch):
        t = orig_tables(arch)
        return {k: (v if k == "sqrt_and_others" else set()) for k, v in
                t.items()}

    F32 = mybir.dt.float32
    BF = mybir.dt.bfloat16
    F8 = mybir.dt.float8e4
    U16 = mybir.dt.uint16
    AL = mybir.AluOpType
    ACT = mybir.ActivationFunctionType

    nc = bacc.Bacc("TRN2", target_bir_lowering=False, debug=False,
                   enable_asserts=False, num_devices=8)
    grp0 = nc.dram_tensor("grp0", [P, 8 * W], BF, kind="ExternalInput").ap()
    grp1 = nc.dram_tensor("grp1", [P, 8 * W], BF, kind="ExternalInput").ap()
    grp2 = nc.dram_tensor("grp2", [P, 13 * W], BF, kind="ExternalInput").ap()
    conf = nc.dram_tensor("conf", [P, CONF_W], F8, kind="ExternalInput").ap()
    gcls = nc.dram_tensor("gcls", [P, 20 * W], F8, kind="ExternalInput").ap()
    out = nc.dram_tensor("acc", [1, 1], F32, kind="ExternalOutput").ap()

    vec, act, gp = nc.vector, nc.scalar, nc.gpsimd

    with tile.TileContext(nc) as tc:
        with (
            tc.tile_pool(name="io", bufs=1) as io,
            tc.tile_pool(name="scr", bufs=1) as scr,
        ):
            # Sqrt first so sqrt_and_others (also carrying square/copy) is
            # the loaded function set, during the DMA window
            eps_t = io.tile([P, 1], F32)
            vec.memset(eps_t[:], 1e-6)
            dum = scr.tile([P, 1], F32, tag="dum")
            act.activation(dum[:], eps_t[:], ACT.Sqrt)

            # ---- loads ----
            g0 = io.tile([P, 8 * W], BF)
            nc.sync.dma_start(out=g0[:], in_=grp0[:])
            g1 = io.tile([P, 8 * W], BF)
            nc.sync.dma_start(out=g1[:], in_=grp1[:])
            g2 = io.tile([P, 13 * W], BF)
            nc.sync.dma_start(out=g2[:], in_=grp2[:])
            gcls_t = io.tile([P, 20 * W], F8)
            nc.sync.dma_start(out=gcls_t[:], in_=gcls[:])
            conf_t = io.tile([P, CONF_W], F8)
            nc.scalar.dma_start(out=conf_t[:], in_=conf[:])

            # acc cols: 0=coordobjA, 1=coordobjB, 2=cls, 3=clsr, 4=conf
            acc = io.tile([P, 7], F32)
            vec.memset(acc[:], 0.0)

            c3 = lambda apv, c: apv.rearrange("p (k c) -> p k c", c=c)
            gwh_f = g0[:, 0:4 * W]
            gxy_f = g0[:, 4 * W:8 * W]
            LT2_f = g1[:, 0:4 * W]
            RB2_f = g1[:, 4 * W:8 * W]
            TXY_f = g2[:, 0:4 * W]
            TSSQ_f = g2[:, 4 * W:8 * W]
            TAB4_f = g2[:, 8 * W:10 * W]
            CLSR = g2[:, 10 * W:11 * W]
            GC = g2[:, 11 * W:13 * W]

            def t(tag, k):
                return scr.tile([P, W * k], BF, tag=tag, name=tag)[:]

            # ---- DVE chain: IoU select mask (flat unit-stride bf16) ----
            hwh_f = t("hwh", 4)
            vec.tensor_scalar_mul(out=hwh_f, in0=gwh_f, scalar1=3.5)
            awh_f = t("awh", 4)
            vec.tensor_scalar(out=awh_f.bitcast(U16),
                              in0=gwh_f.bitcast(U16), scalar1=0x7FFF,
                              scalar2=None, op0=AL.bitwise_and)
            sb_f = t("sb", 4)
            vec.tensor_scalar(out=sb_f.bitcast(U16),
                              in0=gwh_f.bitcast(U16), scalar1=0x8000,
                              scalar2=None, op0=AL.bitwise_and)
            # ACT: sqrt early (ahead of the big conf square) so the
            # GpSimd signed-sqrt finish never gates anyone
            sq4_f = t("sq4", 4)
            act.activation(c3(sq4_f, 4), c3(awh_f, 4), ACT.Sqrt,
                           bias=eps_t[:])
            confsq = scr.tile([P, CONF_W], BF, tag="confsq")
            act.activation(confsq[:], conf_t[:], ACT.Square, scale=RT2I,
                           accum_out=acc[:, 4:5])

            lt_f = t("lt", 4)
            vec.tensor_tensor(out=lt_f, in0=gxy_f, in1=hwh_f, op=AL.subtract)
            rb_f = t("rb", 4)
            vec.tensor_tensor(out=rb_f, in0=gxy_f, in1=hwh_f, op=AL.add)
            m1_f = t("m1", 4)
            vec.tensor_tensor(out=m1_f, in0=rb_f, in1=RB2_f, op=AL.min)
            m2_f = t("m2", 4)
            vec.tensor_tensor(out=m2_f, in0=lt_f, in1=LT2_f, op=AL.max)
            wih_f = t("wih", 4)
            vec.tensor_tensor(out=wih_f, in0=m1_f, in1=m2_f, op=AL.subtract)
            # relu fused with *0.5: the area algebra runs at 1/4 scale
            vec.tensor_scalar(out=wih_f, in0=wih_f, scalar1=0.0, scalar2=0.5,
                              op0=AL.max, op1=AL.mult)
            wih = c3(wih_f, 4)

            ain_f = t("ain", 2)
            vec.tensor_tensor(out=c3(ain_f, 2), in0=wih[:, :, 0:2],
                              in1=wih[:, :, 2:4], op=AL.mult)
            hwh = c3(hwh_f, 4)
            atot_f = t("atot", 2)
            vec.tensor_tensor(out=c3(atot_f, 2), in0=hwh[:, :, 0:2],
                              in1=hwh[:, :, 2:4], op=AL.mult)
            vec.tensor_tensor(out=atot_f, in0=atot_f, in1=ain_f,
                              op=AL.subtract)
            vec.tensor_tensor(out=atot_f, in0=atot_f, in1=TAB4_f, op=AL.add)
            vec.tensor_scalar_max(out=atot_f, in0=atot_f, scalar1=2.5e-7)

            ain = c3(ain_f, 2)
            atot = c3(atot_f, 2)
            c10 = t("c10", 1)
            vec.tensor_tensor(out=c10, in0=ain[:, :, 1], in1=atot[:, :, 0],
                              op=AL.mult)
            c01 = t("c01", 1)
            vec.tensor_tensor(out=c01, in0=ain[:, :, 0], in1=atot[:, :, 1],
                              op=AL.mult)
            msel_f = t("msel", 2)
            msel = c3(msel_f, 2)
            vec.tensor_tensor(out=msel[:, :, 1], in0=c10, in1=c01,
                              op=AL.is_gt)
            vec.tensor_scalar(out=msel[:, :, 0], in0=msel[:, :, 1],
                              scalar1=-1.0, scalar2=1.0, op0=AL.mult,
                              op1=AL.add)

            # ---- d10 = [txy-xy (4), tssq-ssq (4), (c-2)/sqrt10 (2)] ----
            # the sqrt-dependent pieces run on the idle GpSimd so the DVE
            # stream never blocks on the Scalar engine mid-chain
            sgnT_f = t("sgnT", 4)
            vec.tensor_tensor(out=sgnT_f.bitcast(U16),
                              in0=TSSQ_f.bitcast(U16),
                              in1=sb_f.bitcast(U16), op=AL.bitwise_xor)
            d10_f = t("d10", 10)
            d10 = c3(d10_f, 10)
            vec.tensor_tensor(out=d10[:, :, 0:4], in0=c3(TXY_f, 4),
                              in1=c3(gxy_f, 4), op=AL.subtract)
            vec.tensor_tensor(out=d10[:, :, 4:8], in0=c3(sgnT_f, 4),
                              in1=c3(sq4_f, 4), op=AL.subtract)
            vec.tensor_scalar(out=d10[:, :, 8:10], in0=c3(GC, 2),
                              scalar1=-2.0, scalar2=RT10I, op0=AL.add,
                              op1=AL.mult)

            # masked halves (separate tiles so ACT can start on half A
            # while the DVE finishes half B); accum 5*sum(masked^2)
            HW_ = 5 * W
            mselb = msel.unsqueeze(2).to_broadcast([P, W, 5, 2])
            r4 = lambda f: f.rearrange("p (k d b) -> p k d b", d=5, b=2)
            WH = W // 2
            for half, col in ((0, 0), (1, 1)):
                dmh = t(f"dm{half}", 5)
                vec.tensor_tensor(
                    out=dmh.rearrange("p (k d b) -> p k d b", d=5, b=2),
                    in0=r4(d10_f)[:, half * WH:(half + 1) * WH],
                    in1=mselb[:, half * WH:(half + 1) * WH], op=AL.mult)
                dsq = t(f"dsq{half}", 5)
                act.activation(dsq, dmh, ACT.Square, scale=RT5,
                               accum_out=acc[:, col:col + 1])

            # ---- classes ----
            clssq = scr.tile([P, W * 20], BF, tag="clssq")
            act.activation(clssq[:], gcls_t[:], ACT.Square,
                           accum_out=acc[:, 2:3])
            clro = t("clro", 1)
            act.activation(clro, CLSR, ACT.Copy, scale=-2.0,
                           accum_out=acc[:, 3:4])

            # cross-partition reduce on the (idle) GpSimd so the store
            # is one descriptor instead of 128
            accr = io.tile([1, 1], F32)
            gp.tensor_reduce(out=accr[:], in_=acc[:],
                             axis=mybir.AxisListType.XYZWC, op=AL.add)
            nc.sync.dma_start(out=out[:], in_=accr[:])
    bacc.get_activation_tables = _one_set
    try:
        nc.compile()
    finally:
        bacc.get_activation_tables = orig_tables
    return nc


def _get_nc():
    if "nc" not in _cache:
        _cache["nc"] = _build()
    return _cache["nc"]


def _host_prep(output, target):
    f32 = np.float32
    bf16 = ml_dtypes.bfloat16
    fp8 = ml_dtypes.float8_e4m3
    out_flat = output.reshape(CELLS, 30)

    bid = target[:, 7].astype(np.int64)
    gx = target[:, 4].astype(np.int64)
    gy = target[:, 5].astype(np.int64)
    cell = bid * (GRID * GRID) + gx * GRID + gy
    core = cell // CELLS_CORE
    rows = out_flat[cell]                      # [NTGT, 30] gather (host)

    x = target[:, 0].astype(f32)
    y = target[:, 1].astype(f32)
    w_ = target[:, 2].astype(f32)
    h_ = target[:, 3].astype(f32)
    c35 = f32(3.5)
    ssw = np.sign(w_) * np.sqrt(np.abs(w_) + f32(1e-6))
    ssh = np.sign(h_) * np.sqrt(np.abs(h_) + f32(1e-6))
    lef, rig = x - c35 * w_, x + c35 * w_
    top, bot = y - c35 * h_, y + c35 * h_
    area4 = w_ * h_ * f32(49.0 / 4.0)
    clsid = target[:, 6].astype(np.int64)
    clsr_all = rows[np.arange(NTGT), 10 + clsid]

    txy_all = np.stack([x, x, y, y], axis=1)
    tssq_all = np.stack([ssw, ssw, ssh, ssh], axis=1)
    lt_all = np.stack([lef, lef, top, top], axis=1)
    rb_all = np.stack([rig, rig, bot, bot], axis=1)
    conf_all = out_flat[:, 4:10:5]             # [CELLS, 2]

    def slots(arr2d, k, n):
        a = np.zeros((NS, k), dtype=f32)
        a[:n] = arr2d
        return a.reshape(W, P, k).transpose(1, 0, 2).reshape(P, W * k)

    in_maps = []
    for c in range(8):
        m = core == c
        n = int(m.sum())
        assert n <= NS, f"slot overflow: core {c} n={n}"
        r = rows[m]

        grp0 = np.concatenate([
            slots(r[:, [2, 7, 3, 8]], 4, n),       # gwh
            slots(r[:, [0, 5, 1, 6]], 4, n),       # gxy
        ], axis=1).astype(bf16)
        grp1 = np.concatenate([
            slots(lt_all[m], 4, n),
            slots(rb_all[m], 4, n),
        ], axis=1).astype(bf16)
        grp2 = np.concatenate([
            slots(txy_all[m], 4, n),
            slots(tssq_all[m], 4, n),
            slots(area4[m][:, None].repeat(2, axis=1), 2, n),
            slots(clsr_all[m][:, None], 1, n),
            slots(r[:, [4, 9]], 2, n),             # gc
        ], axis=1).astype(bf16)

        gcls = slots(r[:, 10:30], 20, n).astype(fp8)
        confc = np.ascontiguousarray(
            conf_all[c * CELLS_CORE:(c + 1) * CELLS_CORE]).reshape(
                P, CONF_W).astype(fp8)
        in_maps.append({"grp0": grp0, "grp1": grp1, "grp2": grp2,
                        "gcls": gcls, "conf": confc})
    return in_maps


def _reduce(results):
    # loss = sum(partials) - NTGT (obj identity) + NTGT (cls_r identity)
    #        - 2*PAD_TOT (pad obj residue)
    tot = 0.0
    for res in results:
        tot += float(res["acc"].astype(np.float64).sum())
    tot -= 2.0 * PAD_TOT
    return np.float32(tot)


def run(output, target, trace=False, trace_cores=None):
    from concourse.bass_utils import run_bass_kernel_spmd

    nc = _get_nc()
    in_maps = _host_prep(np.asarray(output), np.asarray(target))
    r = run_bass_kernel_spmd(nc, in_maps, core_ids=list(range(8)), trace=trace,
                             trace_cores=trace_cores)
    return _reduce(r.results), r


def kernel(output, target):
    return run(output, target)[0]


# revision 31
# speedup vs baseline: 1.0412x; 1.0412x over previous
"""YOLO-style loss (nn_Loss_90142773608781) on 8 Trainium2 NeuronCores.

Strategy (data-parallel, host-side sharding + gather):
- Cells sharded by batch range: core c owns cells [c*100352, (c+1)*100352).
  Targets follow their cell's core (batch_id // 2048).
- The host gathers each target's 30-float grid row (pure data movement)
  and builds one dense per-core bf16 tile in a dim-major SoA layout
  ([x0,x1,y0,y1], [w0,w1,h0,h1], ...) so every DVE op is unit-stride;
  one big load (4.2KB per-partition lines) instead of many small ones.
  Target-side fields (signed sqrts, box edges, areas/4) are precomputed
  on host and duplicated per box lane to keep packed bf16 DVE modes.
- On device each core runs ONE full-width pass over its 9216 slots
  (72 per partition): IoU cross-multiply box select, then *masked
  accumulation* - every per-target term is computed for BOTH boxes and
  summed with the 0/1 responsibility mask, so there is no box-gather.
  The iou guard drops out: ain>0 implies atot >= area_t >> 1e-6, so only
  u=max(atot,eps) is needed; the whole area algebra runs at 1/4 scale
  (host tab/4, wih relu fused with *0.5) which the cross-multiply
  comparison is invariant to.
- Padding slots are all-zero; their only residue is the obj term's
  0.5*(0-2)^2 = 2 per pad, corrected on host.
- The obj term rides the coord/size Square-accumulate: masked diffs and
  the masked (c-2)/sqrt(10) live in one [P,W,10] layout reduced by
  activation(Square, scale=sqrt(5), accum_out), split in two halves so
  the Scalar engine can start while the DVE finishes the second half.
- abs/sign for the signed sqrt are bf16 bit ops on the DVE (and 0x7fff /
  and 0x8000 + or), keeping ACT to Sqrt/Square/Copy (one table set).
- conf and cls-grid squares ride fp8 (e4m3): random rounding cancels
  across 1.6M/1.3M terms; the ~0.1% systematic square bias is far below
  the 2e-2 gate.
- Host reduces the [P,7] partials; constants: obj identity -NTGT and
  cls_r identity +NTGT cancel; pad obj residue -2*8192 remains.
"""

import sys

if "/opt/trn_rl_repo" not in sys.path:
    sys.path.append("/opt/trn_rl_repo")

import numpy as np
import ml_dtypes

P = 128
W = 64                    # slots per partition (65536/8 targets = 8192 = P*W)
NS = P * W                # 8192 slots per core
GRID = 7
BATCH = 16384
NTGT = 65536
CELLS = BATCH * GRID * GRID
CELLS_CORE = CELLS // 8   # 100352
CONF_W = CELLS_CORE * 2 // P   # 1568

RT5 = 2.2360679774997896   # sqrt(5)
RT2I = 0.7071067811865476  # sqrt(0.5)
RT10I = 0.31622776601683794  # 1/sqrt(10)

# grp1 blocks (units of W columns):
# [gwh 0:4][gxy 4:8][lt 8:12][rb 12:16][txy 16:20][tssq 20:24]
# [tab4 24:26][clsr 26:27][gc 27:29]
G1W = 29 * W

_cache = {}


def _build():
    import concourse.bacc as bacc
    import concourse.tile as tile
    import concourse.mybir as mybir
    from concourse import hw_specs

    # The act-table-load pass picks, per activation, the FIRST set in
    # act_info.json containing its function; square/copy then resolve to
    # set 0 while sqrt needs set 3 -> two ~1.3us table loads. Blanking
    # every set except sqrt_and_others (indices preserved) makes all our
    # functions (sqrt/square/copy live there too) resolve to one set.
    orig_tables = hw_specs.get_activation_tables

    def _one_set(arch):
        t = orig_tables(arch)
        return {k: (v if k == "sqrt_and_others" else set()) for k, v in
                t.items()}

    F32 = mybir.dt.float32
    BF = mybir.dt.bfloat16
    F8 = mybir.dt.float8e4
    U16 = mybir.dt.uint16
    AL = mybir.AluOpType
    ACT = mybir.ActivationFunctionType

    nc = bacc.Bacc("TRN2", target_bir_lowering=False, debug=False,
                   enable_asserts=False, num_devices=8)
    grp0 = nc.dram_tensor("grp0", [P, 16 * W], BF, kind="ExternalInput").ap()
    grp2 = nc.dram_tensor("grp2", [P, 13 * W], BF, kind="ExternalInput").ap()
    conf = nc.dram_tensor("conf", [P, CONF_W], F8, kind="ExternalInput").ap()
    gcls = nc.dram_tensor("gcls", [P, 20 * W], F8, kind="ExternalInput").ap()
    out = nc.dram_tensor("acc", [1, 1], F32, kind="ExternalOutput").ap()

    vec, act, gp = nc.vector, nc.scalar, nc.gpsimd

    with tile.TileContext(nc) as tc:
        with (
            tc.tile_pool(name="io", bufs=1) as io,
            tc.tile_pool(name="scr", bufs=1) as scr,
        ):
            # Sqrt first so sqrt_and_others (also carrying square/copy) is
            # the loaded function set, during the DMA window
            eps_t = io.tile([P, 1], F32)
            vec.memset(eps_t[:], 1e-6)
            dum = scr.tile([P, 1], F32, tag="dum")
            act.activation(dum[:], eps_t[:], ACT.Sqrt)

            # ---- loads ----
            g0 = io.tile([P, 16 * W], BF)
            nc.sync.dma_start(out=g0[:], in_=grp0[:])
            g2 = io.tile([P, 13 * W], BF)
            nc.sync.dma_start(out=g2[:], in_=grp2[:])
            gcls_t = io.tile([P, 20 * W], F8)
            nc.sync.dma_start(out=gcls_t[:], in_=gcls[:])
            conf_t = io.tile([P, CONF_W], F8)
            nc.scalar.dma_start(out=conf_t[:], in_=conf[:])

            # acc cols: 0=coordobjA, 1=coordobjB, 2=cls, 3=clsr, 4=conf
            acc = io.tile([P, 7], F32)
            vec.memset(acc[:], 0.0)

            c3 = lambda apv, c: apv.rearrange("p (k c) -> p k c", c=c)
            gwh_f = g0[:, 0:4 * W]
            gxy_f = g0[:, 4 * W:8 * W]
            LT2_f = g0[:, 8 * W:12 * W]
            RB2_f = g0[:, 12 * W:16 * W]
            TXY_f = g2[:, 0:4 * W]
            TSSQ_f = g2[:, 4 * W:8 * W]
            TAB4_f = g2[:, 8 * W:10 * W]
            CLSR = g2[:, 10 * W:11 * W]
            GC = g2[:, 11 * W:13 * W]

            def t(tag, k):
                return scr.tile([P, W * k], BF, tag=tag, name=tag)[:]

            # ---- DVE chain: IoU select mask (flat unit-stride bf16) ----
            hwh_f = t("hwh", 4)
            vec.tensor_scalar_mul(out=hwh_f, in0=gwh_f, scalar1=3.5)
            awh_f = t("awh", 4)
            vec.tensor_scalar(out=awh_f.bitcast(U16),
                              in0=gwh_f.bitcast(U16), scalar1=0x7FFF,
                              scalar2=None, op0=AL.bitwise_and)
            sb_f = t("sb", 4)
            vec.tensor_scalar(out=sb_f.bitcast(U16),
                              in0=gwh_f.bitcast(U16), scalar1=0x8000,
                              scalar2=None, op0=AL.bitwise_and)
            # ACT: sqrt early (ahead of the big conf square) so the
            # GpSimd signed-sqrt finish never gates anyone
            sq4_f = t("sq4", 4)
            act.activation(c3(sq4_f, 4), c3(awh_f, 4), ACT.Sqrt,
                           bias=eps_t[:])
            confsq = scr.tile([P, CONF_W], BF, tag="confsq")
            act.activation(confsq[:], conf_t[:], ACT.Square, scale=RT2I,
                           accum_out=acc[:, 4:5])

            lt_f = t("lt", 4)
            vec.tensor_tensor(out=lt_f, in0=gxy_f, in1=hwh_f, op=AL.subtract)
            rb_f = t("rb", 4)
            vec.tensor_tensor(out=rb_f, in0=gxy_f, in1=hwh_f, op=AL.add)
            m1_f = t("m1", 4)
            vec.tensor_tensor(out=m1_f, in0=rb_f, in1=RB2_f, op=AL.min)
            m2_f = t("m2", 4)
            vec.tensor_tensor(out=m2_f, in0=lt_f, in1=LT2_f, op=AL.max)
            wih_f = t("wih", 4)
            vec.tensor_tensor(out=wih_f, in0=m1_f, in1=m2_f, op=AL.subtract)
            # relu fused with *0.5: the area algebra runs at 1/4 scale
            vec.tensor_scalar(out=wih_f, in0=wih_f, scalar1=0.0, scalar2=0.5,
                              op0=AL.max, op1=AL.mult)
            wih = c3(wih_f, 4)

            ain_f = t("ain", 2)
            vec.tensor_tensor(out=c3(ain_f, 2), in0=wih[:, :, 0:2],
                              in1=wih[:, :, 2:4], op=AL.mult)
            hwh = c3(hwh_f, 4)
            atot_f = t("atot", 2)
            vec.tensor_tensor(out=c3(atot_f, 2), in0=hwh[:, :, 0:2],
                              in1=hwh[:, :, 2:4], op=AL.mult)
            vec.tensor_tensor(out=atot_f, in0=atot_f, in1=ain_f,
                              op=AL.subtract)
            vec.tensor_tensor(out=atot_f, in0=atot_f, in1=TAB4_f, op=AL.add)
            vec.tensor_scalar_max(out=atot_f, in0=atot_f, scalar1=2.5e-7)

            ain = c3(ain_f, 2)
            atot = c3(atot_f, 2)
            c10 = t("c10", 1)
            vec.tensor_tensor(out=c10, in0=ain[:, :, 1], in1=atot[:, :, 0],
                              op=AL.mult)
            c01 = t("c01", 1)
            vec.tensor_tensor(out=c01, in0=ain[:, :, 0], in1=atot[:, :, 1],
                              op=AL.mult)
            msel_f = t("msel", 2)
            msel = c3(msel_f, 2)
            vec.tensor_tensor(out=msel[:, :, 1], in0=c10, in1=c01,
                              op=AL.is_gt)
            vec.tensor_scalar(out=msel[:, :, 0], in0=msel[:, :, 1],
                              scalar1=-1.0, scalar2=1.0, op0=AL.mult,
                              op1=AL.add)

            # ---- d10 = [txy-xy (4), tssq-ssq (4), (c-2)/sqrt10 (2)] ----
            # the sqrt-dependent pieces run on the idle GpSimd so the DVE
            # stream never blocks on the Scalar engine mid-chain
            sgnT_f = t("sgnT", 4)
            vec.tensor_tensor(out=sgnT_f.bitcast(U16),
                              in0=TSSQ_f.bitcast(U16),
                              in1=sb_f.bitcast(U16), op=AL.bitwise_xor)
            d10_f = t("d10", 10)
            d10 = c3(d10_f, 10)
            vec.tensor_tensor(out=d10[:, :, 0:4], in0=c3(TXY_f, 4),
                              in1=c3(gxy_f, 4), op=AL.subtract)
            vec.tensor_tensor(out=d10[:, :, 4:8], in0=c3(sgnT_f, 4),
                              in1=c3(sq4_f, 4), op=AL.subtract)
            vec.tensor_scalar(out=d10[:, :, 8:10], in0=c3(GC, 2),
                              scalar1=-2.0, scalar2=RT10I, op0=AL.add,
                              op1=AL.mult)

            # masked halves (separate tiles so ACT can start on half A
            # while the DVE finishes half B); accum 5*sum(masked^2)
            HW_ = 5 * W
            mselb = msel.unsqueeze(2).to_broadcast([P, W, 5, 2])
            r4 = lambda f: f.rearrange("p (k d b) -> p k d b", d=5, b=2)
            WH = W // 2
            for half, col in ((0, 0), (1, 1)):
                dmh = t(f"dm{half}", 5)
                vec.tensor_tensor(
                    out=dmh.rearrange("p (k d b) -> p k d b", d=5, b=2),
                    in0=r4(d10_f)[:, half * WH:(half + 1) * WH],
                    in1=mselb[:, half * WH:(half + 1) * WH], op=AL.mult)
                dsq = t(f"dsq{half}", 5)
                act.activation(dsq, dmh, ACT.Square, scale=RT5,
                               accum_out=acc[:, col:col + 1])

            # ---- classes ----
            clssq = scr.tile([P, W * 20], BF, tag="clssq")
            act.activation(clssq[:], gcls_t[:], ACT.Square,
                           accum_out=acc[:, 2:3])
            clro = t("clro", 1)
            act.activation(clro, CLSR, ACT.Copy, scale=-2.0,
                           accum_out=acc[:, 3:4])

            # cross-partition reduce on the (idle) GpSimd so the store
            # is one descriptor instead of 128
            accr = io.tile([1, 1], F32)
            gp.tensor_reduce(out=accr[:], in_=acc[:],
                             axis=mybir.AxisListType.XYZWC, op=AL.add)
            nc.sync.dma_start(out=out[:], in_=accr[:])
    bacc.get_activation_tables = _one_set
    try:
        nc.compile()
    finally:
        bacc.get_activation_tables = orig_tables
    return nc


def _get_nc():
    if "nc" not in _cache:
        _cache["nc"] = _build()
    return _cache["nc"]


def _host_prep(output, target):
    f32 = np.float32
    bf16 = ml_dtypes.bfloat16
    fp8 = ml_dtypes.float8_e4m3
    out_flat = output.reshape(CELLS, 30)

    bid = target[:, 7].astype(np.int64)
    gx = target[:, 4].astype(np.int64)
    gy = target[:, 5].astype(np.int64)
    cell = bid * (GRID * GRID) + gx * GRID + gy
    rows = out_flat[cell]                      # [NTGT, 30] gather (host)

    x = target[:, 0].astype(f32)
    y = target[:, 1].astype(f32)
    w_ = target[:, 2].astype(f32)
    h_ = target[:, 3].astype(f32)
    c35 = f32(3.5)
    ssw = np.sign(w_) * np.sqrt(np.abs(w_) + f32(1e-6))
    ssh = np.sign(h_) * np.sqrt(np.abs(h_) + f32(1e-6))
    lef, rig = x - c35 * w_, x + c35 * w_
    top, bot = y - c35 * h_, y + c35 * h_
    area4 = w_ * h_ * f32(49.0 / 4.0)
    clsid = target[:, 6].astype(np.int64)
    clsr_all = rows[np.arange(NTGT), 10 + clsid]

    txy_all = np.stack([x, x, y, y], axis=1)
    tssq_all = np.stack([ssw, ssw, ssh, ssh], axis=1)
    lt_all = np.stack([lef, lef, top, top], axis=1)
    rb_all = np.stack([rig, rig, bot, bot], axis=1)
    conf_all = out_flat[:, 4:10:5]             # [CELLS, 2]

    def slots(arr2d, k):
        return arr2d.reshape(W, P, k).transpose(1, 0, 2).reshape(P, W * k)

    in_maps = []
    for c in range(8):
        sl = slice(c * NS, (c + 1) * NS)
        r = rows[sl]

        grp0 = np.concatenate([
            slots(r[:, [2, 7, 3, 8]], 4),          # gwh
            slots(r[:, [0, 5, 1, 6]], 4),          # gxy
            slots(lt_all[sl], 4),
            slots(rb_all[sl], 4),
        ], axis=1).astype(bf16)
        grp2 = np.concatenate([
            slots(txy_all[sl], 4),
            slots(tssq_all[sl], 4),
            slots(area4[sl][:, None].repeat(2, axis=1), 2),
            slots(clsr_all[sl][:, None], 1),
            slots(r[:, [4, 9]], 2),                # gc
        ], axis=1).astype(bf16)

        gcls = slots(r[:, 10:30], 20).astype(fp8)
        confc = np.ascontiguousarray(
            conf_all[c * CELLS_CORE:(c + 1) * CELLS_CORE]).reshape(
                P, CONF_W).astype(fp8)
        in_maps.append({"grp0": grp0, "grp2": grp2,
                        "gcls": gcls, "conf": confc})
    return in_maps


def _reduce(results):
    # loss = sum(partials) - NTGT (obj identity) + NTGT (cls_r identity)
    #        - 2*PAD_TOT (pad obj residue)
    # obj identity (-NTGT) and cls_r identity (+NTGT) cancel; no pads.
    tot = 0.0
    for res in results:
        tot += float(res["acc"].astype(np.float64).sum())
    return np.float32(tot)


def run(output, target, trace=False, trace_cores=None):
    from concourse.bass_utils import run_bass_kernel_spmd

    nc = _get_nc()
    in_maps = _host_prep(np.asarray(output), np.asarray(target))
    r = run_bass_kernel_spmd(nc, in_maps, core_ids=list(range(8)), trace=trace,
                             trace_cores=trace_cores)
    return _reduce(r.results), r


def kernel(output, target):
    return run(output, target)[0]
